# revision 1
# baseline (speedup 1.0000x reference)
"""AdaptiveSparseAttention on 8 TRN2 NeuronCores — v2.

Sharding: tensor-parallel over heads (4 heads/core) for QKV+attention,
exact-f32 router via partial matmul + AllReduce, AllToAll reshard to
token-parallel for the output projection. Host gathers 8 token shards.

v2 vs baseline:
 - x transposed + bf16-cast on host, replicated to every core (xT):
   the on-device transpose pass and both AllGathers are gone.
 - weights arrive pre-cast to bf16 in their final SBUF layouts.
 - router softmax/top-4 overlaps the QKV matmuls.
 - attention: prebuilt V transposes, paired 2-bank PSUM tiles (one exp
   per k-block covering both local heads), causal column trimming,
   per-tqt batched epilogue.
"""
import sys
sys.path.insert(0, "/opt/trn_rl_repo")
import numpy as np
import ml_dtypes
import concourse.bass as bass
import concourse.mybir as mybir
import concourse.tile as tile
from concourse import bacc
from concourse.bass_utils import run_bass_kernel_spmd
from concourse.masks import make_identity

DT = mybir.dt
F32 = DT.float32
BF16 = DT.bfloat16
AF = mybir.ActivationFunctionType
OP = mybir.AluOpType

NCORES = 8
B, T, D = 4, 1024, 2048
H, DH = 32, 64
HL = 4             # local heads per core
NTOK = B * T       # 4096 flattened tokens
DSL = D // NCORES  # 256
TB = 512
NTB = NTOK // TB   # 8
KT = D // 128      # 16
ROPE_BASE = 10000.0
BF = ml_dtypes.bfloat16


def _build(debug=False):
    nc = bacc.Bacc("TRN2", target_bir_lowering=False, debug=False, num_devices=NCORES)
    xT = nc.dram_tensor("xT", [D, NTOK], BF16, kind="ExternalInput").ap()
    xsl = nc.dram_tensor("xsl", [DSL, NTOK], F32, kind="ExternalInput").ap()
    wqk = nc.dram_tensor("wqk", [128, KT * 4, 128], BF16, kind="ExternalInput").ap()
    wv = nc.dram_tensor("wv", [128, KT * 2, 128], BF16, kind="ExternalInput").ap()
    wr = nc.dram_tensor("wr", [128, 2, H], F32, kind="ExternalInput").ap()
    wo = nc.dram_tensor("wo", [128, 16, D], BF16, kind="ExternalInput").ap()
    cs = nc.dram_tensor("cs", [128, 2, TB], F32, kind="ExternalInput").ap()
    sn = nc.dram_tensor("sn", [128, 2, TB], F32, kind="ExternalInput").ap()
    sel = nc.dram_tensor("sel", [H, 97], F32, kind="ExternalInput").ap()
    out = nc.dram_tensor("out", [TB, D], F32, kind="ExternalOutput").ap()
    if debug:
        dbg_qkT = nc.dram_tensor("dbg_qkT", [128, 4, NTOK], BF16,
                                 kind="ExternalOutput").ap()
        dbg_gl = nc.dram_tensor("dbg_gl", [97, NTOK], BF16,
                                kind="ExternalOutput").ap()
        dbg_va = nc.dram_tensor("dbg_va", [128, 8, 2, 66], BF16,
                                kind="ExternalOutput").ap()
        dbg_a2a = nc.dram_tensor("dbg_a2a", [1024, TB], BF16,
                                 kind="ExternalOutput").ap()
        dbg_ar = nc.dram_tensor("dbg_ar", [128, 32, H], F32,
                                kind="ExternalOutput").ap()

    xT_r = xT.rearrange("(k p) t -> p k t", p=128)     # [128, KT, NTOK]
    xsl_r = xsl.rearrange("(a p) t -> p a t", p=128)   # [128, 2, NTOK]

    with tile.TileContext(nc) as tc:
        with (
            tc.tile_pool(name="consts", bufs=1) as consts,
            tc.tile_pool(name="persist", bufs=1) as persist,
            tc.tile_pool(name="xrp", bufs=2) as xrp,
            tc.tile_pool(name="vtp", bufs=2) as vtp,
            tc.tile_pool(name="stream", bufs=2) as stream,
            tc.tile_pool(name="rope", bufs=2) as rope,
            tc.tile_pool(name="rt", bufs=1) as rt,
            tc.tile_pool(name="gx", bufs=2) as gx,
            tc.tile_pool(name="att", bufs=2) as att,
            tc.tile_pool(name="epi", bufs=1) as epi,
            tc.tile_pool(name="oproj", bufs=1) as oproj,
            tc.tile_pool(name="oproj2", bufs=2) as oproj2,
            tc.tile_pool(name="ps", bufs=1, space="PSUM") as ps,
            tc.tile_pool(name="dram", bufs=1, space="DRAM") as dram,
        ):
            # ---- consts ----
            ident_f = consts.tile([128, 128], F32)
            make_identity(nc, ident_f[:])
            ident_b = consts.tile([128, 128], BF16)
            make_identity(nc, ident_b[:])
            ones_bc = consts.tile([64, 64], BF16)   # rows 0/32 used as bc lhsT
            nc.vector.memset(ones_bc[:], 1.0)
            sel_sb = consts.tile([H, 97], F32)
            nc.sync.dma_start(sel_sb[:], sel[:])

            # ---- persistent SBUF ----
            qkT_m = persist.tile([128, 4, NTOK], BF16)
            qkTb = [qkT_m[:, :, _b * T:(_b + 1) * T] for _b in range(B)]
            gate_l = persist.tile([97, NTOK], BF16)    # rows 32*l = local heads

            wqk_sb = persist.tile([128, KT * 4, 128], BF16)
            for _c in range(8):
                nc.sync.dma_start(wqk_sb[:, _c * 8:(_c + 1) * 8, :],
                                  wqk[:, _c * 8:(_c + 1) * 8, :])
            wv_sb = persist.tile([128, KT * 2, 128], BF16)
            for _c in range(4):
                nc.sync.dma_start(wv_sb[:, _c * 8:(_c + 1) * 8, :],
                                  wv[:, _c * 8:(_c + 1) * 8, :])
            wr_sb = persist.tile([128, 2, H], F32)
            nc.sync.dma_start(wr_sb[:], wr[:])
            cs_sb = persist.tile([128, 2, TB], F32)
            nc.sync.dma_start(cs_sb[:], cs[:])
            sn_sb = persist.tile([128, 2, TB], F32)
            nc.sync.dma_start(sn_sb[:], sn[:])

            # va tiles: [tok-partition, tkb, hl, 64 dh + ones col]
            va_t = {}
            for hp in range(2):
                for b in range(B):
                    va_t[(hp, b)] = persist.tile([128, 8, 2, 66], BF16,
                                                 name=f"va{hp}_{b}")

            # ---- DRAM internal ----
            ar_in = dram.tile([128, 32, H], F32)
            ar_out = dram.tile([128, 32, H], F32)
            a2a_in = [dram.tile([1024, TB], BF16, name=f"a2a_in{_i}") for _i in range(2)]
            a2a_out = [dram.tile([1024, TB], BF16, name=f"a2a_out{_i}") for _i in range(2)]

            # ---- router partials (exact f32) + AllReduce ----
            def router_partials():
              for g in range(8):
                xs = xrp.tile([128, 2, 4 * 128], F32, tag="xs")
                for _a in range(2):
                    nc.sync.dma_start(xs[:, _a, :], xsl_r[:, _a, g * TB:(g + 1) * TB])
                ps_r = ps.tile([128, 4, H], F32, tag="T3", name="ps_r")
                for j in range(4):
                    for a in range(2):
                        nc.tensor.matmul(ps_r[:, j, :], xs[:, a, j * 128:(j + 1) * 128],
                                         wr_sb[:, a, :], start=(a == 0), stop=(a == 1))
                rsb = xrp.tile([128, 4, H], F32, tag="rsb")
                nc.vector.tensor_copy(rsb[:], ps_r[:])
                nc.sync.dma_start(ar_in[:, g * 4:(g + 1) * 4, :], rsb[:])
              nc.gpsimd.collective_compute(
                "AllReduce", OP.add, replica_groups=[list(range(NCORES))],
                ins=[ar_in.opt()], outs=[ar_out.opt()])

            # ---- router softmax + top-4 + local gate rows (overlaps QKV) ----
            e = persist.tile([128, 32, H], F32)

            def phase3():
              for _c in range(2):
                nc.sync.dma_start(e[:, _c * 16:(_c + 1) * 16, :],
                                  ar_out[:, _c * 16:(_c + 1) * 16, :])
              rmax = rt.tile([128, 32], F32, tag="rmax")
              nc.vector.tensor_reduce(rmax[:], e[:], axis=mybir.AxisListType.X, op=OP.max)
              nc.vector.tensor_tensor(e[:], e[:], rmax[:, :, None].to_broadcast((128, 32, H)),
                                      OP.subtract)
              nc.scalar.activation(e[:].rearrange("p a h -> p (a h)"),
                                   e[:].rearrange("p a h -> p (a h)"), AF.Exp)
              ssum = rt.tile([128, 32], F32, tag="ssum")
              nc.vector.tensor_reduce(ssum[:], e[:], axis=mybir.AxisListType.X, op=OP.add)
              rs = rt.tile([128, 32], F32, tag="rs")
              nc.vector.reciprocal(rs[:], ssum[:])
              ecur = rt.tile([128, 32, H], F32, tag="ecur")
              nc.vector.tensor_copy(ecur[:], e[:])
              ge = rt.tile([128, 32, H], BF16, tag="geb", name="ge")
              for it in range(4):
                m = rt.tile([128, 32], F32, tag="m")
                nc.vector.tensor_reduce(m[:], ecur[:], axis=mybir.AxisListType.X, op=OP.max)
                nc.vector.tensor_tensor(ge[:], ecur[:],
                                        m[:, :, None].to_broadcast((128, 32, H)), OP.is_ge)
                nc.vector.scalar_tensor_tensor(ecur[:], ge[:], -1e9, ecur[:],
                                               OP.mult, OP.add)
              mask = rt.tile([128, 32, H], BF16, tag="geb", name="mask")
              nc.vector.tensor_scalar(mask[:], ecur[:], -1e6, None, OP.is_lt)
              gate = e
              nc.vector.tensor_tensor(gate[:], e[:], mask[:], OP.mult)
              nc.vector.tensor_tensor(gate[:], gate[:],
                                      rs[:, :, None].to_broadcast((128, 32, H)), OP.mult)
              # gate_l[l, tok] = gate[tok, 4c+l] via transpose + sel matmul
              for tokb in range(32):
                gt_ps = ps.tile([H, 128], F32, tag="T4", name="gt_ps")
                nc.tensor.transpose(gt_ps[:], gate[:, tokb, :], ident_f[:])
                gt_sb = gx.tile([H, 128], F32, tag="gt_sb")
                nc.vector.tensor_copy(gt_sb[:], gt_ps[:])
                g4_ps = ps.tile([97, 128], F32, tag="T3", name="g4_ps")
                nc.tensor.matmul(g4_ps[:], sel_sb[:], gt_sb[:], start=True, stop=True)
                nc.vector.tensor_copy(gate_l[:, tokb * 128:(tokb + 1) * 128], g4_ps[:])

            # ---- QKV pass: qkv^T tiles + RoPE; va transposes per batch ----
            for tb in range(NTB):
                b = tb // 2
                half = tb % 2
                tsl = slice(tb * TB, (tb + 1) * TB)
                vT = vtp.tile([128, 2, TB], BF16, tag="vT")
                rh = []
                for hh in range(2):
                    r = stream.tile([128, 8, TB], BF16, tag="rhs", name=f"rhs{hh}")
                    for _c in range(4):
                        nc.sync.dma_start(
                            r[:, _c * 2:(_c + 1) * 2, :],
                            xT_r[:, hh * 8 + _c * 2:hh * 8 + (_c + 1) * 2, tsl])
                    rh.append(r)
                ps_qk = [ps.tile([128, 2, TB], F32, tag=f"T{q}", name=f"ps_qk{q}")
                         for q in range(2)]
                ps_v = ps.tile([128, 2, TB], F32, tag="T2", name="ps_v")
                for kt in range(KT):
                    rhs = rh[kt // 8][:, kt % 8, :]
                    for cb in range(4):
                        nc.tensor.matmul(ps_qk[cb // 2][:, cb % 2, :],
                                         wqk_sb[:, kt * 4 + cb, :],
                                         rhs, start=(kt == 0), stop=(kt == KT - 1))
                    for vb in range(2):
                        nc.tensor.matmul(ps_v[:, vb, :], wv_sb[:, kt * 2 + vb, :],
                                         rhs, start=(kt == 0), stop=(kt == KT - 1))
                # RoPE epilogue on the 4 qk blocks
                for cb in range(4):
                    src = ps_qk[cb // 2][:, cb % 2, :]
                    csb = rope.tile([128, TB], BF16, tag="C")
                    nc.scalar.activation(csb[:], src, AF.Copy)
                    swp = rope.tile([128, TB], BF16, tag="S")
                    for gsw in range(4):
                        sg = gsw ^ 1
                        nc.sync.dma_start(swp[gsw * 32:(gsw + 1) * 32, :],
                                          csb[sg * 32:(sg + 1) * 32, :])
                    t1 = rope.tile([128, TB], F32, tag="T1r")
                    nc.vector.tensor_tensor(t1[:], src, cs_sb[:, half, :], OP.mult)
                    t2 = rope.tile([128, TB], F32, tag="T2r")
                    nc.gpsimd.tensor_tensor(t2[:], swp[:], sn_sb[:, half, :], OP.mult)
                    nc.vector.tensor_tensor(
                        qkTb[b][:, cb, half * TB:(half + 1) * TB],
                        t1[:], t2[:], OP.add)
                for vb in range(2):
                    nc.scalar.activation(vT[:, vb, :], ps_v[:, vb, :], AF.Copy)
                # build va slices for this token tile (both head-pairs)
                for hp in range(2):
                    va = va_t[(hp, b)]
                    if half == 0:
                        nc.vector.memset(va[:, :, :, 64:65], 1.0)
                    for hl in range(2):
                        base = hl * 64
                        idn = ident_b[base:base + 64, base:base + 64]
                        for tk2 in range(4):
                            vps = ps.tile([128, 64], BF16, tag="T4", name="vps")
                            nc.tensor.transpose(
                                vps[:], vT[base:base + 64, hp,
                                           tk2 * 128:(tk2 + 1) * 128], idn)
                            nc.vector.tensor_copy(
                                va[:, half * 4 + tk2, hl, 0:64], vps[:])
                if tb == 0:
                    router_partials()
                if tb == 2:
                    phase3()

            # ---- attention + AllToAll + out-projection ----
            def attention(hp):
                for b in range(B):
                    qkT = qkTb[b]
                    va = va_t[(hp, b)]
                    o_t0 = ps.tile([128, 2, TB], F32, tag="T2", name="o_t0")
                    o_t1 = [ps.tile([65, TB], F32, tag=f"T{3 + _h}", name=f"o_t1{_h}")
                            for _h in range(2)]
                    o_ap = [lambda hl, sl, _o=o_t0: _o[0:65, hl, sl],
                            lambda hl, sl, _o=o_t1: _o[hl][0:65, sl]]
                    for tqt in range(2):
                        lq = tqt * TB              # local (within batch)
                        gq = b * T + tqt * TB      # global token index
                        ntk = 4 + 4 * tqt
                        for tkb in range(ntk):
                            ksl = slice(tkb * 128, (tkb + 1) * 128)
                            dd = tqt * 4 - tkb
                            off = 0 if dd >= 1 else -dd * 128
                            s_pair = ps.tile([128, 2, TB], F32, tag=f"T{tkb % 2}",
                                             name="s_pair")
                            nc.tensor.matmul(s_pair[:, 0, off:TB],
                                             qkT[0:64, 2 + hp, ksl],
                                             qkT[0:64, hp, lq + off:lq + TB],
                                             start=True, stop=True,
                                             tile_position=(0, 0))
                            nc.tensor.matmul(s_pair[:, 1, off:TB],
                                             qkT[64:128, 2 + hp, ksl],
                                             qkT[64:128, hp, lq + off:lq + TB],
                                             start=True, stop=True,
                                             tile_position=(64, 0))
                            p_pair = att.tile([128, 2, TB], BF16, tag="pp",
                                              name="p_pair")
                            nc.scalar.activation(p_pair[:, :, off:TB],
                                                 s_pair[:, :, off:TB],
                                                 AF.Exp, scale=0.125)
                            if dd < 1:
                                nc.gpsimd.affine_select(
                                    out=p_pair[:, :, off:off + 128],
                                    in_=p_pair[:, :, off:off + 128],
                                    compare_op=OP.is_ge, fill=0.0,
                                    base=0, pattern=[[0, 2], [1, 128]],
                                    channel_multiplier=-1)
                            for hl in range(2):
                                nc.tensor.matmul(o_ap[tqt](hl, slice(off, TB)),
                                                 va[:, tkb, hl, 0:65],
                                                 p_pair[:, hl, off:TB],
                                                 start=(tkb == 0),
                                                 stop=(tkb == ntk - 1))
                        # epilogue for (hp, b, tqt): normalize + gate (at p0)
                        j = 2 * b + tqt
                        for hl in range(2):
                            o_full = o_t0[:, hl, :] if tqt == 0 else o_t1[hl][:, :]
                            dnh = epi.tile([1, TB], F32, tag=f"dn{hl}",
                                           name=f"dn{hl}")
                            nc.vector.tensor_copy(dnh[:], o_full[64:65, :])
                            nc.vector.reciprocal_approx_fast(dnh[:], dnh[:])
                            grh = epi.tile([1, TB], BF16, tag=f"gr{hl}",
                                           name=f"gr{hl}")
                            nc.vector.tensor_copy(
                                grh[:],
                                gate_l[32 * (2 * hp + hl):32 * (2 * hp + hl) + 1,
                                       gq:gq + TB])
                            sch = epi.tile([1, TB], BF16, tag=f"sc{hl}",
                                           name=f"sc{hl}")
                            nc.vector.tensor_tensor(sch[:], dnh[:], grh[:], OP.mult)
                            bc_sb = att.tile([64, TB], BF16, tag=f"bcs{hl}",
                                             name=f"bcs{hl}")
                            nc.gpsimd.partition_broadcast(bc_sb[:], sch[:])
                            oT = att.tile([64, TB], BF16, tag=f"oT{hl}",
                                          name=f"oT{hl}")
                            nc.vector.tensor_tensor(oT[:], o_full[0:64, :],
                                                    bc_sb[:], OP.mult)
                            nc.sync.dma_start(
                                a2a_in[hp][j * 128 + hl * 64:j * 128 + (hl + 1) * 64, :],
                                oT[:])

            def outproj(hp):
                rcv = [oproj.tile([128, TB], BF16, tag=f"rcv{i}", name=f"rcv{i}")
                       for i in range(8)]
                for i in range(8):
                    nc.sync.dma_start(rcv[i][:], a2a_out[hp][i * 128:(i + 1) * 128, :])
                for nh in range(4):
                    wo_sb = oproj2.tile([128, 8, TB], BF16, tag="wo")
                    for _i in range(8):
                        nc.sync.dma_start(wo_sb[:, _i, :],
                                          wo[:, hp * 8 + _i, nh * TB:(nh + 1) * TB])
                    ops_t = [ps.tile([128, 2, TB], F32, tag=f"T{q}",
                                     name=f"op{q}") for q in range(2)]
                    for i in range(8):
                        for m_ in range(4):
                            nc.tensor.matmul(
                                ops_t[m_ // 2][:, m_ % 2, :],
                                rcv[i][:, m_ * 128:(m_ + 1) * 128],
                                wo_sb[:, i, :],
                                start=(i == 0), stop=(i == 7))
                    for m_ in range(4):
                        ostage = oproj2.tile([128, TB], F32, tag="ostage")
                        nc.vector.tensor_copy(ostage[:], ops_t[m_ // 2][:, m_ % 2, :])
                        r0 = slice(m_ * 128, (m_ + 1) * 128)
                        c0 = slice(nh * TB, (nh + 1) * TB)
                        if hp == 0:
                            nc.sync.dma_start(out[r0, c0], ostage[:])
                        else:
                            nc.gpsimd.dma_start(out[r0, c0], ostage[:],
                                                accum_op=OP.add)

            attention(0)
            nc.gpsimd.collective_compute(
                "AllToAll", OP.bypass, replica_groups=[list(range(NCORES))],
                ins=[a2a_in[0].opt()], outs=[a2a_out[0].opt()])
            attention(1)
            nc.gpsimd.collective_compute(
                "AllToAll", OP.bypass, replica_groups=[list(range(NCORES))],
                ins=[a2a_in[1].opt()], outs=[a2a_out[1].opt()])
            outproj(0)
            outproj(1)
            if debug:
                nc.sync.dma_start(dbg_qkT[:], qkT_m[:])
                nc.sync.dma_start(dbg_gl[:], gate_l[:])
                nc.sync.dma_start(dbg_va[:], va_t[(0, 0)][:])
                nc.sync.dma_start(dbg_a2a[:], a2a_in[0][:])
                nc.sync.dma_start(dbg_ar[:], e[:])

    nc.compile()
    return nc


_CACHE = {}


def _get_nc():
    if "nc" not in _CACHE:
        _CACHE["nc"] = _build()
    return _CACHE["nc"]


def _host_inputs(x, w_router, w_qkv, w_out):
    x2 = np.ascontiguousarray(np.asarray(x, dtype=np.float32).reshape(NTOK, D))
    w_qkv = np.asarray(w_qkv, dtype=np.float32)
    w_router = np.asarray(w_router, dtype=np.float32)
    w_out = np.asarray(w_out, dtype=np.float32)

    xT_host = np.ascontiguousarray(x2.T.astype(BF))          # [D, NTOK] bf16

    # RoPE tables (per-batch period; first 1024 tokens cover all)
    invf = 1.0 / (ROPE_BASE ** (np.arange(0, DH, 2, dtype=np.float32) / DH))
    tt = np.arange(T, dtype=np.float32)
    ang = tt[None, :] * invf[:, None]                        # [32, T]
    cos1 = np.cos(ang).astype(np.float32)
    sin1 = np.sin(ang).astype(np.float32)
    cos4 = np.tile(cos1, (4, 1))                             # [128, T]
    ssin4 = np.concatenate([-sin1, sin1, -sin1, sin1], axis=0)
    cs_host = np.ascontiguousarray(cos4.reshape(128, 2, TB))
    sn_host = np.ascontiguousarray(ssin4.reshape(128, 2, TB))

    # w_out layout: (p, hp*8+i, n) = w_out[i*256 + hp*128 + p, n]
    wo_host = np.ascontiguousarray(
        w_out.reshape(8, 2, 128, D).transpose(2, 1, 0, 3).reshape(128, 16, D)
        .astype(BF))

    in_maps = []
    for c in range(NCORES):
        heads = [4 * c + i for i in range(HL)]

        def deint(h, base):
            cols = np.arange(h * DH, (h + 1) * DH)
            return np.concatenate([base + cols[0::2], base + cols[1::2]])

        qk_cols = np.concatenate(
            [deint(heads[0], 0), deint(heads[1], 0),
             deint(heads[2], 0), deint(heads[3], 0),
             deint(heads[0], D), deint(heads[1], D),
             deint(heads[2], D), deint(heads[3], D)])
        v_cols = np.concatenate([2 * D + np.arange(h * DH, (h + 1) * DH)
                                 for h in heads])
        # [2048, 512] -> [128, 64, 128]
        wqk_host = np.ascontiguousarray(
            w_qkv[:, qk_cols].reshape(KT, 128, 4, 128)
            .transpose(1, 0, 2, 3).reshape(128, KT * 4, 128).astype(BF))
        wv_host = np.ascontiguousarray(
            w_qkv[:, v_cols].reshape(KT, 128, 2, 128)
            .transpose(1, 0, 2, 3).reshape(128, KT * 2, 128).astype(BF))
        wr_host = np.ascontiguousarray(
            w_router[c * DSL:(c + 1) * DSL, :].reshape(2, 128, H)
            .transpose(1, 0, 2))
        sel_np = np.zeros((H, 97), dtype=np.float32)
        for l in range(HL):
            sel_np[4 * c + l, 32 * l] = 1.0
        in_maps.append({
            "xT": xT_host,
            "xsl": np.ascontiguousarray(x2[:, c * DSL:(c + 1) * DSL].T),
            "wqk": wqk_host,
            "wv": wv_host,
            "wr": wr_host,
            "wo": wo_host,
            "cs": cs_host,
            "sn": sn_host,
            "sel": sel_np,
        })
    return in_maps


def run(x, w_router, w_qkv, w_out, trace=False, debug=False):
    if debug:
        nc = _build(debug=True)
    else:
        nc = _get_nc()
    in_maps = _host_inputs(x, w_router, w_qkv, w_out)
    res = run_bass_kernel_spmd(nc, in_maps, core_ids=list(range(NCORES)), trace=trace)
    shards = [res.results[c]["out"] for c in range(NCORES)]
    full = np.concatenate(shards, axis=0).reshape(B, T, D).astype(np.float32)
    return full, res


def kernel(x, w_router, w_qkv, w_out):
    full, _ = run(x, w_router, w_qkv, w_out, trace=False)
    return full



# revision 34
# speedup vs baseline: 1.1214x; 1.1214x over previous
"""AdaptiveSparseAttention on 8 TRN2 NeuronCores — v2.

Sharding: tensor-parallel over heads (4 heads/core) for QKV+attention,
exact-f32 router via partial matmul + AllReduce, AllToAll reshard to
token-parallel for the output projection. Host gathers 8 token shards.

v2 vs baseline:
 - x transposed + bf16-cast on host, replicated to every core (xT):
   the on-device transpose pass and both AllGathers are gone.
 - weights arrive pre-cast to bf16 in their final SBUF layouts.
 - router softmax/top-4 overlaps the QKV matmuls.
 - attention: prebuilt V transposes, paired 2-bank PSUM tiles (one exp
   per k-block covering both local heads), causal column trimming,
   per-tqt batched epilogue.
"""
import sys
sys.path.insert(0, "/opt/trn_rl_repo")
import numpy as np
import ml_dtypes
import concourse.bass as bass
import concourse.mybir as mybir
import concourse.tile as tile
from concourse import bacc
from concourse.bass_utils import run_bass_kernel_spmd
from concourse.masks import make_identity

DT = mybir.dt
F32 = DT.float32
BF16 = DT.bfloat16
AF = mybir.ActivationFunctionType
OP = mybir.AluOpType

NCORES = 8
B, T, D = 4, 1024, 2048
H, DH = 32, 64
HL = 4             # local heads per core
NTOK = B * T       # 4096 flattened tokens
DSL = D // NCORES  # 256
TB = 512
NTB = NTOK // TB   # 8
KT = D // 128      # 16
ROPE_BASE = 10000.0
BF = ml_dtypes.bfloat16


def _build(debug=False):
    nc = bacc.Bacc("TRN2", target_bir_lowering=False, debug=False, num_devices=NCORES)
    xT = nc.dram_tensor("xT", [NTB, 128, KT, TB], BF16,
                        kind="ExternalInput").ap()
    xsl = nc.dram_tensor("xsl", [DSL, NTOK], F32, kind="ExternalInput").ap()
    wqk = nc.dram_tensor("wqk", [128, KT * 4, 128], BF16, kind="ExternalInput").ap()
    wv = nc.dram_tensor("wv", [128, KT, 256], BF16,
                        kind="ExternalInput").ap()
    wr = nc.dram_tensor("wr", [128, 2, H], F32, kind="ExternalInput").ap()
    wo = nc.dram_tensor("wo", [2, 4, 128, 8, TB], BF16,
                        kind="ExternalInput").ap()
    cs = nc.dram_tensor("cs", [128, 2, TB], BF16, kind="ExternalInput").ap()
    sn = nc.dram_tensor("sn", [128, 2, TB], BF16, kind="ExternalInput").ap()
    sel = nc.dram_tensor("sel", [H, 97], F32, kind="ExternalInput").ap()
    out = nc.dram_tensor("out", [TB, D], BF16, kind="ExternalOutput").ap()
    if debug:
        dbg_qkT = nc.dram_tensor("dbg_qkT", [128, 4, NTOK], BF16,
                                 kind="ExternalOutput").ap()
        dbg_gl = nc.dram_tensor("dbg_gl", [97, NTOK], BF16,
                                kind="ExternalOutput").ap()
        dbg_va = nc.dram_tensor("dbg_va", [128, 8, 2, 66], BF16,
                                kind="ExternalOutput").ap()
        dbg_a2a = nc.dram_tensor("dbg_a2a", [1024, TB], BF16,
                                 kind="ExternalOutput").ap()
        dbg_ar = nc.dram_tensor("dbg_ar", [128, 32, H], F32,
                                kind="ExternalOutput").ap()

    xsl_r = xsl.rearrange("(a p) t -> p a t", p=128)   # [128, 2, NTOK]

    with tile.TileContext(nc) as tc:
        with (
            tc.tile_pool(name="consts", bufs=1) as consts,
            tc.tile_pool(name="persist", bufs=1) as persist,
            tc.tile_pool(name="xrp", bufs=2) as xrp,
            tc.tile_pool(name="vtp", bufs=2) as vtp,
            tc.tile_pool(name="stream", bufs=2) as stream,
            tc.tile_pool(name="rope", bufs=2) as rope,
            tc.tile_pool(name="rt", bufs=1) as rt,
            tc.tile_pool(name="gx", bufs=2) as gx,
            tc.tile_pool(name="att", bufs=2) as att,
            tc.tile_pool(name="epi", bufs=1) as epi,
            tc.tile_pool(name="oproj", bufs=1) as oproj,
            tc.tile_pool(name="wop", bufs=2) as wop,
            tc.tile_pool(name="ps", bufs=1, space="PSUM") as ps,
            tc.tile_pool(name="dram", bufs=1, space="DRAM") as dram,
        ):
            # ---- consts ----
            ident_f = consts.tile([128, 128], F32)
            make_identity(nc, ident_f[:])
            sel_sb = consts.tile([H, 97], F32)
            nc.scalar.dma_start(sel_sb[:], sel[:])

            # ---- persistent SBUF ----
            qkT_m = persist.tile([128, 4, NTOK], BF16)
            qkTb = [qkT_m[:, :, _b * T:(_b + 1) * T] for _b in range(B)]
            gate_l = persist.tile([97, NTOK], BF16)    # rows 32*l = local heads

            wqk_sb = persist.tile([128, KT * 4, 128], BF16)
            for _c in range(8):
                nc.scalar.dma_start(wqk_sb[:, _c * 8:(_c + 1) * 8, :],
                                    wqk[:, _c * 8:(_c + 1) * 8, :])
            wv_sb = persist.tile([128, KT, 256], BF16)
            for _c in range(2):
                nc.scalar.dma_start(wv_sb[:, _c * 8:(_c + 1) * 8, :],
                                    wv[:, _c * 8:(_c + 1) * 8, :])
            wr_sb = persist.tile([128, 2, H], F32)
            nc.scalar.dma_start(wr_sb[:], wr[:])
            cs_sb = persist.tile([128, 2, TB], BF16)
            nc.scalar.dma_start(cs_sb[:], cs[:])
            sn_sb = persist.tile([128, 2, TB], BF16)
            nc.scalar.dma_start(sn_sb[:], sn[:])

            ostage = persist.tile([128, 16, TB], BF16)

            # va tiles: [tok-partition, tkb, hl, 64 dh + ones col]
            va_t = {}
            for hp in range(2):
                for b in range(B):
                    va_t[(hp, b)] = persist.tile([128, 8, 2, 66], BF16,
                                                 name=f"va{hp}_{b}")

            # ---- DRAM internal ----
            ar_in = dram.tile([128, 32, H], F32)
            ar_out = dram.tile([128, 32, H], F32)
            a2a_in = [dram.tile([1024, TB], BF16, name=f"a2a_in{_i}") for _i in range(2)]
            a2a_out = [dram.tile([1024, TB], BF16, name=f"a2a_out{_i}") for _i in range(2)]

            # ---- router partials (exact f32) + AllReduce ----
            def router_partials():
              for g in range(8):
                xs = xrp.tile([128, 2, 4 * 128], F32, tag="xs", bufs=1)
                for _a in range(2):
                    nc.scalar.dma_start(xs[:, _a, :],
                                        xsl_r[:, _a, g * TB:(g + 1) * TB])
                ps_r = ps.tile([128, 4, H], F32, tag="T3", name="ps_r")
                for j in range(4):
                    for a in range(2):
                        nc.tensor.matmul(ps_r[:, j, :], xs[:, a, j * 128:(j + 1) * 128],
                                         wr_sb[:, a, :], start=(a == 0), stop=(a == 1))
                rsb = xrp.tile([128, 4, H], F32, tag="rsb")
                nc.vector.tensor_copy(rsb[:], ps_r[:])
                nc.sync.dma_start(ar_in[:, g * 4:(g + 1) * 4, :], rsb[:])
              nc.gpsimd.collective_compute(
                "AllReduce", OP.add, replica_groups=[list(range(NCORES))],
                ins=[ar_in.opt()], outs=[ar_out.opt()])

            # ---- router softmax + top-4 + local gate rows (overlaps QKV) ----
            e = persist.tile([128, 32, H], F32)

            def phase3():
              for _c in range(2):
                nc.sync.dma_start(e[:, _c * 16:(_c + 1) * 16, :],
                                  ar_out[:, _c * 16:(_c + 1) * 16, :])
              rmax = rt.tile([128, 32], F32, tag="rmax")
              nc.vector.tensor_reduce(rmax[:], e[:], axis=mybir.AxisListType.X, op=OP.max)
              nc.vector.tensor_tensor(e[:], e[:], rmax[:, :, None].to_broadcast((128, 32, H)),
                                      OP.subtract)
              nc.scalar.activation(e[:].rearrange("p a h -> p (a h)"),
                                   e[:].rearrange("p a h -> p (a h)"), AF.Exp)
              ssum = rt.tile([128, 32], F32, tag="ssum")
              nc.vector.tensor_reduce(ssum[:], e[:], axis=mybir.AxisListType.X, op=OP.add)
              rs = rt.tile([128, 32], F32, tag="rs")
              nc.vector.reciprocal(rs[:], ssum[:])
              ecur = rt.tile([128, 32, H], F32, tag="ecur")
              nc.vector.tensor_copy(ecur[:], e[:])
              ge = rt.tile([128, 32, H], BF16, tag="geb", name="ge")
              for it in range(4):
                m = rt.tile([128, 32], F32, tag="m")
                nc.vector.tensor_reduce(m[:], ecur[:], axis=mybir.AxisListType.X, op=OP.max)
                nc.vector.tensor_tensor(ge[:], ecur[:],
                                        m[:, :, None].to_broadcast((128, 32, H)), OP.is_ge)
                nc.vector.scalar_tensor_tensor(ecur[:], ge[:], -1e9, ecur[:],
                                               OP.mult, OP.add)
              mask = rt.tile([128, 32, H], BF16, tag="geb", name="mask")
              nc.vector.tensor_scalar(mask[:], ecur[:], -1e6, None, OP.is_lt)
              gate = e
              nc.vector.tensor_tensor(gate[:], e[:], mask[:], OP.mult)
              nc.vector.tensor_tensor(gate[:], gate[:],
                                      rs[:, :, None].to_broadcast((128, 32, H)), OP.mult)
              # gate_l[l, tok] = gate[tok, 4c+l] via transpose + sel matmul
              for tokb in range(32):
                gt_ps = ps.tile([H, 128], F32, tag="T4", name="gt_ps")
                nc.tensor.transpose(gt_ps[:], gate[:, tokb, :], ident_f[:])
                gt_sb = gx.tile([H, 128], F32, tag="gt_sb")
                nc.vector.tensor_copy(gt_sb[:], gt_ps[:])
                g4_ps = ps.tile([97, 128], F32, tag="T3", name="g4_ps")
                nc.tensor.matmul(g4_ps[:], sel_sb[:], gt_sb[:], start=True, stop=True)
                nc.vector.tensor_copy(gate_l[:, tokb * 128:(tokb + 1) * 128], g4_ps[:])

            wo_t = {}

            def _wo_chunk(hp, nh):
                wt = wop.tile([128, 8, TB], BF16, tag="wo",
                              name=f"wo{hp}_{nh}")
                nc.scalar.dma_start(wt[:], wo[hp, nh, :, :, :])
                wo_t[(hp, nh)] = wt

            # ---- QKV pass: qkv^T tiles + RoPE; va transposes per batch ----
            for tb in range(NTB):
                b = tb // 2
                half = tb % 2
                tsl = slice(tb * TB, (tb + 1) * TB)
                rh = []
                for hh in range(2):
                    r = stream.tile([128, 8, TB], BF16, tag=f"rhs{hh}",
                                    name=f"rhs{hh}")
                    nc.sync.dma_start(r[:], xT[tb, :, hh * 8:(hh + 1) * 8, :])
                    rh.append(r)
                ps_qk = [ps.tile([128, 2, TB], F32, tag=f"T{q}", name=f"ps_qk{q}")
                         for q in range(2)]
                for kt in range(KT):
                    rhs = rh[kt // 8][:, kt % 8, :]
                    for cb in range(4):
                        nc.tensor.matmul(ps_qk[cb // 2][:, cb % 2, :],
                                         wqk_sb[:, kt * 4 + cb, :],
                                         rhs, start=(kt == 0), stop=(kt == KT - 1))
                # RoPE epilogue, batched per qk tile (2 cbs each)
                for q in range(2):
                    csb = rope.tile([128, 2, TB], BF16, tag="C", bufs=1)
                    nc.scalar.activation(
                        csb[:].rearrange("p a t -> p (a t)"),
                        ps_qk[q][:].rearrange("p a t -> p (a t)"), AF.Copy)
                    swp = rope.tile([128, 2, TB], BF16, tag="S", bufs=1)
                    for gsw in range(4):
                        sg = gsw ^ 1
                        nc.sync.dma_start(swp[gsw * 32:(gsw + 1) * 32, :, :],
                                          csb[sg * 32:(sg + 1) * 32, :, :])
                    for jj in range(2):
                        cb = q * 2 + jj
                        t1 = rope.tile([128, TB], F32, tag="T1r", bufs=1)
                        nc.vector.tensor_tensor(t1[:], csb[:, jj, :],
                                                cs_sb[:, half, :], OP.mult)
                        t2 = rope.tile([128, TB], F32, tag="T2r", bufs=1)
                        nc.gpsimd.tensor_tensor(t2[:], swp[:, jj, :],
                                                sn_sb[:, half, :], OP.mult)
                        nc.vector.tensor_tensor(
                            qkTb[b][:, cb, half * TB:(half + 1) * TB],
                            t1[:], t2[:], OP.add)
                # v-group: x stationary -> V in [tok, dh] layout directly
                vg = ps.tile([128, 4, 256], F32, tag="T2", name="vg")
                for kt in range(KT):
                    for sub in range(4):
                        # start only once per PSUM bank (start clears the
                        # whole bank's has_written bits)
                        nc.tensor.matmul(
                            vg[:, sub, :],
                            rh[kt // 8][:, kt % 8, sub * 128:(sub + 1) * 128],
                            wv_sb[:, kt, :],
                            start=(kt == 0 and sub % 2 == 0),
                            stop=(kt == KT - 1))
                for hp in range(2):
                    va = va_t[(hp, b)]
                    if half == 0:
                        nc.vector.memset(va[:, :, :, 64:65], 1.0)
                    for sub in range(4):
                        for hl in range(2):
                            hh = 2 * hp + hl
                            nc.vector.tensor_copy(
                                va[:, half * 4 + sub, hl, 0:64],
                                vg[:, sub, hh * 64:(hh + 1) * 64])
                if tb == 0:
                    router_partials()
                if tb == 2:
                    phase3()
                if 4 <= tb <= 5:
                    _wo_chunk(0, tb - 4)

            # ---- attention + AllToAll + out-projection ----
            def attention(hp):
                for b in range(B):
                    qkT = qkTb[b]
                    va = va_t[(hp, b)]
                    o_t0 = ps.tile([128, 2, TB], F32, tag="T2", name="o_t0")
                    o_t1 = [ps.tile([65, TB], F32, tag=f"T{3 + _h}", name=f"o_t1{_h}")
                            for _h in range(2)]
                    o_ap = [lambda hl, sl, _o=o_t0: _o[0:65, hl, sl],
                            lambda hl, sl, _o=o_t1: _o[hl][0:65, sl]]
                    for tqt in range(2):
                        lq = tqt * TB              # local (within batch)
                        gq = b * T + tqt * TB      # global token index
                        ntk = 4 + 4 * tqt
                        for tkb in range(ntk):
                            ksl = slice(tkb * 128, (tkb + 1) * 128)
                            dd = tqt * 4 - tkb
                            off = 0 if dd >= 1 else -dd * 128
                            s_pair = ps.tile([128, 2, TB], F32, tag=f"T{tkb % 2}",
                                             name="s_pair")
                            nc.tensor.matmul(s_pair[:, 0, off:TB],
                                             qkT[0:64, 2 + hp, ksl],
                                             qkT[0:64, hp, lq + off:lq + TB],
                                             start=True, stop=True,
                                             tile_position=(0, 0))
                            nc.tensor.matmul(s_pair[:, 1, off:TB],
                                             qkT[64:128, 2 + hp, ksl],
                                             qkT[64:128, hp, lq + off:lq + TB],
                                             start=True, stop=True,
                                             tile_position=(64, 0))
                            p_pair = att.tile([128, 2, TB], BF16, tag="pp",
                                              name="p_pair")
                            nc.scalar.activation(p_pair[:, :, off:TB],
                                                 s_pair[:, :, off:TB],
                                                 AF.Exp, scale=0.125)
                            if dd < 1:
                                nc.gpsimd.affine_select(
                                    out=p_pair[:, :, off:off + 128],
                                    in_=p_pair[:, :, off:off + 128],
                                    compare_op=OP.is_ge, fill=0.0,
                                    base=0, pattern=[[0, 2], [1, 128]],
                                    channel_multiplier=-1)
                            for hl in range(2):
                                nc.tensor.matmul(o_ap[tqt](hl, slice(off, TB)),
                                                 va[:, tkb, hl, 0:65],
                                                 p_pair[:, hl, off:TB],
                                                 start=(tkb == 0),
                                                 stop=(tkb == ntk - 1))
                        # epilogue for (hp, b, tqt): normalize + gate (at p0)
                        j = 2 * b + tqt
                        for hl in range(2):
                            o_full = o_t0[:, hl, :] if tqt == 0 else o_t1[hl][:, :]
                            dnh = epi.tile([1, TB], F32, tag=f"dn{hl}",
                                           name=f"dn{hl}")
                            nc.vector.tensor_copy(dnh[:], o_full[64:65, :])
                            nc.vector.reciprocal_approx_fast(dnh[:], dnh[:])
                            grh = epi.tile([1, TB], BF16, tag=f"gr{hl}",
                                           name=f"gr{hl}")
                            nc.vector.tensor_copy(
                                grh[:],
                                gate_l[32 * (2 * hp + hl):32 * (2 * hp + hl) + 1,
                                       gq:gq + TB])
                            sch = epi.tile([1, TB], BF16, tag=f"sc{hl}",
                                           name=f"sc{hl}")
                            nc.vector.tensor_tensor(sch[:], dnh[:], grh[:], OP.mult)
                            bc_sb = att.tile([64, TB], BF16, tag=f"bcs{hl}",
                                             name=f"bcs{hl}")
                            nc.gpsimd.partition_broadcast(bc_sb[:], sch[:])
                            oT = att.tile([64, TB], BF16, tag=f"oT{hl}",
                                          name=f"oT{hl}")
                            nc.vector.tensor_tensor(oT[:], o_full[0:64, :],
                                                    bc_sb[:], OP.mult)
                            nc.sync.dma_start(
                                a2a_in[hp][j * 128 + hl * 64:j * 128 + (hl + 1) * 64, :],
                                oT[:])

            def outproj(hp):
                rcv = [oproj.tile([128, TB], BF16, tag=f"rcv{i}", name=f"rcv{i}")
                       for i in range(8)]
                for i in range(8):
                    nc.sync.dma_start(rcv[i][:], a2a_out[hp][i * 128:(i + 1) * 128, :])
                for nh in range(4):
                    ops_t = [ps.tile([128, 2, TB], F32, tag=f"T{q}",
                                     name=f"op{q}") for q in range(2)]
                    for i in range(8):
                        for m_ in range(4):
                            nc.tensor.matmul(
                                ops_t[m_ // 2][:, m_ % 2, :],
                                rcv[i][:, m_ * 128:(m_ + 1) * 128],
                                wo_t[(hp, nh)][:, i, :],
                                start=(i == 0), stop=(i == 7))
                    for m_ in range(4):
                        src_ = ops_t[m_ // 2][:, m_ % 2, :]
                        if hp == 0:
                            nc.vector.tensor_copy(
                                ostage[:, nh * 4 + m_, :], src_)
                        else:
                            o2 = oproj.tile([128, TB], BF16, tag="o2", bufs=2,
                                            name="o2")
                            nc.vector.tensor_tensor(
                                o2[:], src_, ostage[:, nh * 4 + m_, :],
                                OP.add)
                            nc.gpsimd.dma_start(
                                out[m_ * 128:(m_ + 1) * 128,
                                    nh * TB:(nh + 1) * TB], o2[:])

            attention(0)
            nc.gpsimd.collective_compute(
                "AllToAll", OP.bypass, replica_groups=[list(range(NCORES))],
                ins=[a2a_in[0].opt()], outs=[a2a_out[0].opt()])
            attention(1)
            nc.gpsimd.collective_compute(
                "AllToAll", OP.bypass, replica_groups=[list(range(NCORES))],
                ins=[a2a_in[1].opt()], outs=[a2a_out[1].opt()])
            _wo_chunk(0, 2)
            _wo_chunk(0, 3)
            outproj(0)
            for nh in range(4):
                _wo_chunk(1, nh)
            outproj(1)
            if debug:
                nc.sync.dma_start(dbg_qkT[:], qkT_m[:])
                nc.sync.dma_start(dbg_gl[:], gate_l[:])
                nc.sync.dma_start(dbg_va[:], va_t[(0, 0)][:])
                nc.sync.dma_start(dbg_a2a[:], a2a_in[0][:])
                nc.sync.dma_start(dbg_ar[:], e[:])

    nc.compile()
    return nc


_CACHE = {}


def _get_nc():
    if "nc" not in _CACHE:
        _CACHE["nc"] = _build()
    return _CACHE["nc"]


def _host_inputs(x, w_router, w_qkv, w_out):
    x2 = np.ascontiguousarray(np.asarray(x, dtype=np.float32).reshape(NTOK, D))
    w_qkv = np.asarray(w_qkv, dtype=np.float32)
    w_router = np.asarray(w_router, dtype=np.float32)
    w_out = np.asarray(w_out, dtype=np.float32)

    xT_d = x2.T.astype(BF)                                   # [D, NTOK]
    xT_host = np.ascontiguousarray(
        xT_d.reshape(KT, 128, NTB, TB).transpose(2, 1, 0, 3))

    # RoPE tables (per-batch period; first 1024 tokens cover all)
    invf = 1.0 / (ROPE_BASE ** (np.arange(0, DH, 2, dtype=np.float32) / DH))
    tt = np.arange(T, dtype=np.float32)
    ang = tt[None, :] * invf[:, None]                        # [32, T]
    cos1 = np.cos(ang).astype(np.float32)
    sin1 = np.sin(ang).astype(np.float32)
    cos4 = np.tile(cos1, (4, 1))                             # [128, T]
    ssin4 = np.concatenate([-sin1, sin1, -sin1, sin1], axis=0)
    cs_host = np.ascontiguousarray(cos4.reshape(128, 2, TB).astype(BF))
    sn_host = np.ascontiguousarray(ssin4.reshape(128, 2, TB).astype(BF))

    # w_out layout: (hp, nh, p, i, t) = w_out[i*256 + hp*128 + p, nh*TB+t]
    wo_host = np.ascontiguousarray(
        w_out.reshape(8, 2, 128, 4, TB).transpose(1, 3, 2, 0, 4).astype(BF))

    in_maps = []
    for c in range(NCORES):
        heads = [4 * c + i for i in range(HL)]

        def deint(h, base):
            cols = np.arange(h * DH, (h + 1) * DH)
            return np.concatenate([base + cols[0::2], base + cols[1::2]])

        qk_cols = np.concatenate(
            [deint(heads[0], 0), deint(heads[1], 0),
             deint(heads[2], 0), deint(heads[3], 0),
             deint(heads[0], D), deint(heads[1], D),
             deint(heads[2], D), deint(heads[3], D)])
        v_cols = np.concatenate([2 * D + np.arange(h * DH, (h + 1) * DH)
                                 for h in heads])
        # [2048, 512] -> [128, 64, 128]
        wqk_host = np.ascontiguousarray(
            w_qkv[:, qk_cols].reshape(KT, 128, 4, 128)
            .transpose(1, 0, 2, 3).reshape(128, KT * 4, 128).astype(BF))
        wv_host = np.ascontiguousarray(
            w_qkv[:, v_cols].reshape(KT, 128, 256)
            .transpose(1, 0, 2).astype(BF))
        wr_host = np.ascontiguousarray(
            w_router[c * DSL:(c + 1) * DSL, :].reshape(2, 128, H)
            .transpose(1, 0, 2))
        sel_np = np.zeros((H, 97), dtype=np.float32)
        for l in range(HL):
            sel_np[4 * c + l, 32 * l] = 1.0
        in_maps.append({
            "xT": xT_host,
            "xsl": np.ascontiguousarray(x2[:, c * DSL:(c + 1) * DSL].T),
            "wqk": wqk_host,
            "wv": wv_host,
            "wr": wr_host,
            "wo": wo_host,
            "cs": cs_host,
            "sn": sn_host,
            "sel": sel_np,
        })
    return in_maps


def run(x, w_router, w_qkv, w_out, trace=False, debug=False):
    if debug:
        nc = _build(debug=True)
    else:
        nc = _get_nc()
    in_maps = _host_inputs(x, w_router, w_qkv, w_out)
    res = run_bass_kernel_spmd(nc, in_maps, core_ids=list(range(NCORES)), trace=trace)
    shards = [res.results[c]["out"] for c in range(NCORES)]
    full = np.concatenate(shards, axis=0).reshape(B, T, D).astype(np.float32)
    return full, res


def kernel(x, w_router, w_qkv, w_out):
    full, _ = run(x, w_router, w_qkv, w_out, trace=False)
    return full



# revision 35
# speedup vs baseline: 1.1241x; 1.0024x over previous
"""AdaptiveSparseAttention on 8 TRN2 NeuronCores — v2.

Sharding: tensor-parallel over heads (4 heads/core) for QKV+attention,
exact-f32 router via partial matmul + AllReduce, AllToAll reshard to
token-parallel for the output projection. Host gathers 8 token shards.

v2 vs baseline:
 - x transposed + bf16-cast on host, replicated to every core (xT):
   the on-device transpose pass and both AllGathers are gone.
 - weights arrive pre-cast to bf16 in their final SBUF layouts.
 - router softmax/top-4 overlaps the QKV matmuls.
 - attention: prebuilt V transposes, paired 2-bank PSUM tiles (one exp
   per k-block covering both local heads), causal column trimming,
   per-tqt batched epilogue.
"""
import sys
sys.path.insert(0, "/opt/trn_rl_repo")
import numpy as np
import ml_dtypes
import concourse.bass as bass
import concourse.mybir as mybir
import concourse.tile as tile
from concourse import bacc
from concourse.bass_utils import run_bass_kernel_spmd
from concourse.masks import make_identity

DT = mybir.dt
F32 = DT.float32
BF16 = DT.bfloat16
AF = mybir.ActivationFunctionType
OP = mybir.AluOpType

NCORES = 8
B, T, D = 4, 1024, 2048
H, DH = 32, 64
HL = 4             # local heads per core
NTOK = B * T       # 4096 flattened tokens
DSL = D // NCORES  # 256
TB = 512
NTB = NTOK // TB   # 8
KT = D // 128      # 16
ROPE_BASE = 10000.0
BF = ml_dtypes.bfloat16


def _build(debug=False):
    nc = bacc.Bacc("TRN2", target_bir_lowering=False, debug=False, num_devices=NCORES)
    xT = nc.dram_tensor("xT", [NTB, 128, KT, TB], BF16,
                        kind="ExternalInput").ap()
    xsl = nc.dram_tensor("xsl", [DSL, NTOK], F32, kind="ExternalInput").ap()
    wqk = nc.dram_tensor("wqk", [128, KT * 4, 128], BF16, kind="ExternalInput").ap()
    wv = nc.dram_tensor("wv", [128, KT, 256], BF16,
                        kind="ExternalInput").ap()
    wr = nc.dram_tensor("wr", [128, 2, H], F32, kind="ExternalInput").ap()
    wo = nc.dram_tensor("wo", [2, 4, 128, 8, TB], BF16,
                        kind="ExternalInput").ap()
    cs = nc.dram_tensor("cs", [128, 2, TB], BF16, kind="ExternalInput").ap()
    sn = nc.dram_tensor("sn", [128, 2, TB], BF16, kind="ExternalInput").ap()
    sel = nc.dram_tensor("sel", [H, 97], F32, kind="ExternalInput").ap()
    out = nc.dram_tensor("out", [TB, D], BF16, kind="ExternalOutput").ap()
    if debug:
        dbg_qkT = nc.dram_tensor("dbg_qkT", [128, 4, NTOK], BF16,
                                 kind="ExternalOutput").ap()
        dbg_gl = nc.dram_tensor("dbg_gl", [97, NTOK], BF16,
                                kind="ExternalOutput").ap()
        dbg_va = nc.dram_tensor("dbg_va", [128, 8, 2, 66], BF16,
                                kind="ExternalOutput").ap()
        dbg_a2a = nc.dram_tensor("dbg_a2a", [1024, TB], BF16,
                                 kind="ExternalOutput").ap()
        dbg_ar = nc.dram_tensor("dbg_ar", [128, 32, H], F32,
                                kind="ExternalOutput").ap()

    xsl_r = xsl.rearrange("(a p) t -> p a t", p=128)   # [128, 2, NTOK]

    with tile.TileContext(nc) as tc:
        with (
            tc.tile_pool(name="consts", bufs=1) as consts,
            tc.tile_pool(name="persist", bufs=1) as persist,
            tc.tile_pool(name="xrp", bufs=2) as xrp,
            tc.tile_pool(name="vtp", bufs=2) as vtp,
            tc.tile_pool(name="stream", bufs=2) as stream,
            tc.tile_pool(name="rope", bufs=2) as rope,
            tc.tile_pool(name="rt", bufs=1) as rt,
            tc.tile_pool(name="gx", bufs=2) as gx,
            tc.tile_pool(name="att", bufs=2) as att,
            tc.tile_pool(name="epi", bufs=1) as epi,
            tc.tile_pool(name="oproj", bufs=1) as oproj,
            tc.tile_pool(name="wop", bufs=2) as wop,
            tc.tile_pool(name="ps", bufs=1, space="PSUM") as ps,
            tc.tile_pool(name="dram", bufs=1, space="DRAM") as dram,
        ):
            # ---- consts ----
            ident_f = consts.tile([128, 128], F32)
            make_identity(nc, ident_f[:])
            sel_sb = consts.tile([H, 97], F32)
            nc.scalar.dma_start(sel_sb[:], sel[:])

            # ---- persistent SBUF ----
            qkT_m = persist.tile([128, 4, NTOK], BF16)
            qkTb = [qkT_m[:, :, _b * T:(_b + 1) * T] for _b in range(B)]
            gate_l = persist.tile([97, NTOK], BF16)    # rows 32*l = local heads

            wqk_sb = persist.tile([128, KT * 4, 128], BF16)
            for _c in range(8):
                nc.scalar.dma_start(wqk_sb[:, _c * 8:(_c + 1) * 8, :],
                                    wqk[:, _c * 8:(_c + 1) * 8, :])
            wv_sb = persist.tile([128, KT, 256], BF16)
            for _c in range(2):
                nc.scalar.dma_start(wv_sb[:, _c * 8:(_c + 1) * 8, :],
                                    wv[:, _c * 8:(_c + 1) * 8, :])
            wr_sb = persist.tile([128, 2, H], F32)
            nc.scalar.dma_start(wr_sb[:], wr[:])
            cs_sb = persist.tile([128, 2, TB], BF16)
            nc.scalar.dma_start(cs_sb[:], cs[:])
            sn_sb = persist.tile([128, 2, TB], BF16)
            nc.scalar.dma_start(sn_sb[:], sn[:])

            ostage = persist.tile([128, 16, TB], BF16)

            # va tiles: [tok-partition, tkb, hl, 64 dh + ones col]
            va_t = {}
            for hp in range(2):
                for b in range(B):
                    va_t[(hp, b)] = persist.tile([128, 8, 2, 66], BF16,
                                                 name=f"va{hp}_{b}")

            # ---- DRAM internal ----
            ar_in = dram.tile([128, 32, H], F32)
            ar_out = dram.tile([128, 32, H], F32)
            a2a_in = [dram.tile([1024, TB], BF16, name=f"a2a_in{_i}") for _i in range(2)]
            a2a_out = [dram.tile([1024, TB], BF16, name=f"a2a_out{_i}") for _i in range(2)]

            # ---- router partials (exact f32) + AllReduce ----
            def router_partials():
              for g in range(8):
                xs = xrp.tile([128, 2, 4 * 128], F32, tag="xs", bufs=1)
                for _a in range(2):
                    nc.scalar.dma_start(xs[:, _a, :],
                                        xsl_r[:, _a, g * TB:(g + 1) * TB])
                ps_r = ps.tile([128, 4, H], F32, tag="T3", name="ps_r")
                for j in range(4):
                    for a in range(2):
                        nc.tensor.matmul(ps_r[:, j, :], xs[:, a, j * 128:(j + 1) * 128],
                                         wr_sb[:, a, :], start=(a == 0), stop=(a == 1))
                rsb = xrp.tile([128, 4, H], F32, tag="rsb")
                nc.vector.tensor_copy(rsb[:], ps_r[:])
                nc.sync.dma_start(ar_in[:, g * 4:(g + 1) * 4, :], rsb[:])
              nc.gpsimd.collective_compute(
                "AllReduce", OP.add, replica_groups=[list(range(NCORES))],
                ins=[ar_in.opt()], outs=[ar_out.opt()])

            # ---- router softmax + top-4 + local gate rows (overlaps QKV) ----
            e = persist.tile([128, 32, H], F32)

            def phase3():
              for _c in range(2):
                nc.sync.dma_start(e[:, _c * 16:(_c + 1) * 16, :],
                                  ar_out[:, _c * 16:(_c + 1) * 16, :])
              rmax = rt.tile([128, 32], F32, tag="rmax")
              nc.vector.tensor_reduce(rmax[:], e[:], axis=mybir.AxisListType.X, op=OP.max)
              nc.vector.tensor_tensor(e[:], e[:], rmax[:, :, None].to_broadcast((128, 32, H)),
                                      OP.subtract)
              nc.scalar.activation(e[:].rearrange("p a h -> p (a h)"),
                                   e[:].rearrange("p a h -> p (a h)"), AF.Exp)
              ssum = rt.tile([128, 32], F32, tag="ssum")
              nc.vector.tensor_reduce(ssum[:], e[:], axis=mybir.AxisListType.X, op=OP.add)
              rs = rt.tile([128, 32], F32, tag="rs")
              nc.vector.reciprocal(rs[:], ssum[:])
              ecur = rt.tile([128, 32, H], F32, tag="ecur")
              nc.vector.tensor_copy(ecur[:], e[:])
              ge = rt.tile([128, 32, H], BF16, tag="geb", name="ge")
              for it in range(4):
                m = rt.tile([128, 32], F32, tag="m")
                nc.vector.tensor_reduce(m[:], ecur[:], axis=mybir.AxisListType.X, op=OP.max)
                nc.vector.tensor_tensor(ge[:], ecur[:],
                                        m[:, :, None].to_broadcast((128, 32, H)), OP.is_ge)
                nc.vector.scalar_tensor_tensor(ecur[:], ge[:], -1e9, ecur[:],
                                               OP.mult, OP.add)
              mask = rt.tile([128, 32, H], BF16, tag="geb", name="mask")
              nc.vector.tensor_scalar(mask[:], ecur[:], -1e6, None, OP.is_lt)
              gate = e
              nc.vector.tensor_tensor(gate[:], e[:], mask[:], OP.mult)
              nc.vector.tensor_tensor(gate[:], gate[:],
                                      rs[:, :, None].to_broadcast((128, 32, H)), OP.mult)
              # gate_l[l, tok] = gate[tok, 4c+l] via transpose + sel matmul
              for tokb in range(32):
                gt_ps = ps.tile([H, 128], F32, tag="T4", name="gt_ps")
                nc.tensor.transpose(gt_ps[:], gate[:, tokb, :], ident_f[:])
                gt_sb = gx.tile([H, 128], F32, tag="gt_sb")
                nc.vector.tensor_copy(gt_sb[:], gt_ps[:])
                g4_ps = ps.tile([97, 128], F32, tag="T3", name="g4_ps")
                nc.tensor.matmul(g4_ps[:], sel_sb[:], gt_sb[:], start=True, stop=True)
                nc.vector.tensor_copy(gate_l[:, tokb * 128:(tokb + 1) * 128], g4_ps[:])

            wo_t = {}

            def _wo_chunk(hp, nh):
                wt = wop.tile([128, 8, TB], BF16, tag="wo",
                              name=f"wo{hp}_{nh}")
                nc.scalar.dma_start(wt[:], wo[hp, nh, :, :, :])
                wo_t[(hp, nh)] = wt

            # ---- QKV pass: qkv^T tiles + RoPE; va transposes per batch ----
            for tb in range(NTB):
                b = tb // 2
                half = tb % 2
                tsl = slice(tb * TB, (tb + 1) * TB)
                rh = []
                for hh in range(2):
                    r = stream.tile([128, 8, TB], BF16, tag=f"rhs{hh}",
                                    name=f"rhs{hh}")
                    for hq in range(2):
                        nc.sync.dma_start(
                            r[:, hq * 4:(hq + 1) * 4, :],
                            xT[tb, :, hh * 8 + hq * 4:hh * 8 + (hq + 1) * 4, :])
                    rh.append(r)
                ps_qk = [ps.tile([128, 2, TB], F32, tag=f"T{q}", name=f"ps_qk{q}")
                         for q in range(2)]
                for kt in range(KT):
                    rhs = rh[kt // 8][:, kt % 8, :]
                    for cb in range(4):
                        nc.tensor.matmul(ps_qk[cb // 2][:, cb % 2, :],
                                         wqk_sb[:, kt * 4 + cb, :],
                                         rhs, start=(kt == 0), stop=(kt == KT - 1))
                # RoPE epilogue, batched per qk tile (2 cbs each)
                for q in range(2):
                    csb = rope.tile([128, 2, TB], BF16, tag="C", bufs=1)
                    nc.scalar.activation(
                        csb[:].rearrange("p a t -> p (a t)"),
                        ps_qk[q][:].rearrange("p a t -> p (a t)"), AF.Copy)
                    swp = rope.tile([128, 2, TB], BF16, tag="S", bufs=1)
                    for gsw in range(4):
                        sg = gsw ^ 1
                        nc.sync.dma_start(swp[gsw * 32:(gsw + 1) * 32, :, :],
                                          csb[sg * 32:(sg + 1) * 32, :, :])
                    for jj in range(2):
                        cb = q * 2 + jj
                        t1 = rope.tile([128, TB], F32, tag="T1r", bufs=1)
                        nc.vector.tensor_tensor(t1[:], csb[:, jj, :],
                                                cs_sb[:, half, :], OP.mult)
                        t2 = rope.tile([128, TB], F32, tag="T2r", bufs=1)
                        nc.gpsimd.tensor_tensor(t2[:], swp[:, jj, :],
                                                sn_sb[:, half, :], OP.mult)
                        nc.vector.tensor_tensor(
                            qkTb[b][:, cb, half * TB:(half + 1) * TB],
                            t1[:], t2[:], OP.add)
                # v-group: x stationary -> V in [tok, dh] layout directly
                vg = ps.tile([128, 4, 256], F32, tag="T2", name="vg")
                for kt in range(KT):
                    for sub in range(4):
                        # start only once per PSUM bank (start clears the
                        # whole bank's has_written bits)
                        nc.tensor.matmul(
                            vg[:, sub, :],
                            rh[kt // 8][:, kt % 8, sub * 128:(sub + 1) * 128],
                            wv_sb[:, kt, :],
                            start=(kt == 0 and sub % 2 == 0),
                            stop=(kt == KT - 1))
                for hp in range(2):
                    va = va_t[(hp, b)]
                    if half == 0:
                        nc.vector.memset(va[:, :, :, 64:65], 1.0)
                    for sub in range(4):
                        for hl in range(2):
                            hh = 2 * hp + hl
                            nc.vector.tensor_copy(
                                va[:, half * 4 + sub, hl, 0:64],
                                vg[:, sub, hh * 64:(hh + 1) * 64])
                if tb == 0:
                    router_partials()
                if tb == 5:
                    phase3()
                if 4 <= tb <= 5:
                    _wo_chunk(0, tb - 4)

            # ---- attention + AllToAll + out-projection ----
            def attention(hp):
                for b in range(B):
                    qkT = qkTb[b]
                    va = va_t[(hp, b)]
                    o_t0 = ps.tile([128, 2, TB], F32, tag="T2", name="o_t0")
                    o_t1 = [ps.tile([65, TB], F32, tag=f"T{3 + _h}", name=f"o_t1{_h}")
                            for _h in range(2)]
                    o_ap = [lambda hl, sl, _o=o_t0: _o[0:65, hl, sl],
                            lambda hl, sl, _o=o_t1: _o[hl][0:65, sl]]
                    for tqt in range(2):
                        lq = tqt * TB              # local (within batch)
                        gq = b * T + tqt * TB      # global token index
                        ntk = 4 + 4 * tqt
                        for tkb in range(ntk):
                            ksl = slice(tkb * 128, (tkb + 1) * 128)
                            dd = tqt * 4 - tkb
                            off = 0 if dd >= 1 else -dd * 128
                            s_pair = ps.tile([128, 2, TB], F32, tag=f"T{tkb % 2}",
                                             name="s_pair")
                            nc.tensor.matmul(s_pair[:, 0, off:TB],
                                             qkT[0:64, 2 + hp, ksl],
                                             qkT[0:64, hp, lq + off:lq + TB],
                                             start=True, stop=True,
                                             tile_position=(0, 0))
                            nc.tensor.matmul(s_pair[:, 1, off:TB],
                                             qkT[64:128, 2 + hp, ksl],
                                             qkT[64:128, hp, lq + off:lq + TB],
                                             start=True, stop=True,
                                             tile_position=(64, 0))
                            p_pair = att.tile([128, 2, TB], BF16, tag="pp",
                                              name="p_pair")
                            nc.scalar.activation(p_pair[:, :, off:TB],
                                                 s_pair[:, :, off:TB],
                                                 AF.Exp, scale=0.125)
                            if dd < 1:
                                nc.gpsimd.affine_select(
                                    out=p_pair[:, :, off:off + 128],
                                    in_=p_pair[:, :, off:off + 128],
                                    compare_op=OP.is_ge, fill=0.0,
                                    base=0, pattern=[[0, 2], [1, 128]],
                                    channel_multiplier=-1)
                            for hl in range(2):
                                nc.tensor.matmul(o_ap[tqt](hl, slice(off, TB)),
                                                 va[:, tkb, hl, 0:65],
                                                 p_pair[:, hl, off:TB],
                                                 start=(tkb == 0),
                                                 stop=(tkb == ntk - 1))
                        # epilogue for (hp, b, tqt): normalize + gate (at p0)
                        j = 2 * b + tqt
                        for hl in range(2):
                            o_full = o_t0[:, hl, :] if tqt == 0 else o_t1[hl][:, :]
                            dnh = epi.tile([1, TB], F32, tag=f"dn{hl}",
                                           name=f"dn{hl}")
                            nc.vector.tensor_copy(dnh[:], o_full[64:65, :])
                            nc.vector.reciprocal_approx_fast(dnh[:], dnh[:])
                            grh = epi.tile([1, TB], BF16, tag=f"gr{hl}",
                                           name=f"gr{hl}")
                            nc.vector.tensor_copy(
                                grh[:],
                                gate_l[32 * (2 * hp + hl):32 * (2 * hp + hl) + 1,
                                       gq:gq + TB])
                            sch = epi.tile([1, TB], BF16, tag=f"sc{hl}",
                                           name=f"sc{hl}")
                            nc.vector.tensor_tensor(sch[:], dnh[:], grh[:], OP.mult)
                            bc_sb = att.tile([64, TB], BF16, tag=f"bcs{hl}",
                                             name=f"bcs{hl}")
                            nc.gpsimd.partition_broadcast(bc_sb[:], sch[:])
                            oT = att.tile([64, TB], BF16, tag=f"oT{hl}",
                                          name=f"oT{hl}")
                            nc.vector.tensor_tensor(oT[:], o_full[0:64, :],
                                                    bc_sb[:], OP.mult)
                            nc.sync.dma_start(
                                a2a_in[hp][j * 128 + hl * 64:j * 128 + (hl + 1) * 64, :],
                                oT[:])

            def outproj(hp):
                rcv = [oproj.tile([128, TB], BF16, tag=f"rcv{i}", name=f"rcv{i}")
                       for i in range(8)]
                for i in range(8):
                    nc.sync.dma_start(rcv[i][:], a2a_out[hp][i * 128:(i + 1) * 128, :])
                for nh in range(4):
                    ops_t = [ps.tile([128, 2, TB], F32, tag=f"T{q}",
                                     name=f"op{q}") for q in range(2)]
                    for i in range(8):
                        for m_ in range(4):
                            nc.tensor.matmul(
                                ops_t[m_ // 2][:, m_ % 2, :],
                                rcv[i][:, m_ * 128:(m_ + 1) * 128],
                                wo_t[(hp, nh)][:, i, :],
                                start=(i == 0), stop=(i == 7))
                    for m_ in range(4):
                        src_ = ops_t[m_ // 2][:, m_ % 2, :]
                        if hp == 0:
                            nc.vector.tensor_copy(
                                ostage[:, nh * 4 + m_, :], src_)
                        else:
                            o2 = oproj.tile([128, TB], BF16, tag="o2", bufs=2,
                                            name="o2")
                            nc.vector.tensor_tensor(
                                o2[:], src_, ostage[:, nh * 4 + m_, :],
                                OP.add)
                            nc.gpsimd.dma_start(
                                out[m_ * 128:(m_ + 1) * 128,
                                    nh * TB:(nh + 1) * TB], o2[:])

            attention(0)
            nc.gpsimd.collective_compute(
                "AllToAll", OP.bypass, replica_groups=[list(range(NCORES))],
                ins=[a2a_in[0].opt()], outs=[a2a_out[0].opt()])
            attention(1)
            nc.gpsimd.collective_compute(
                "AllToAll", OP.bypass, replica_groups=[list(range(NCORES))],
                ins=[a2a_in[1].opt()], outs=[a2a_out[1].opt()])
            _wo_chunk(0, 2)
            _wo_chunk(0, 3)
            outproj(0)
            for nh in range(4):
                _wo_chunk(1, nh)
            outproj(1)
            if debug:
                nc.sync.dma_start(dbg_qkT[:], qkT_m[:])
                nc.sync.dma_start(dbg_gl[:], gate_l[:])
                nc.sync.dma_start(dbg_va[:], va_t[(0, 0)][:])
                nc.sync.dma_start(dbg_a2a[:], a2a_in[0][:])
                nc.sync.dma_start(dbg_ar[:], e[:])

    nc.compile()
    return nc


_CACHE = {}


def _get_nc():
    if "nc" not in _CACHE:
        _CACHE["nc"] = _build()
    return _CACHE["nc"]


def _host_inputs(x, w_router, w_qkv, w_out):
    x2 = np.ascontiguousarray(np.asarray(x, dtype=np.float32).reshape(NTOK, D))
    w_qkv = np.asarray(w_qkv, dtype=np.float32)
    w_router = np.asarray(w_router, dtype=np.float32)
    w_out = np.asarray(w_out, dtype=np.float32)

    xT_d = x2.T.astype(BF)                                   # [D, NTOK]
    xT_host = np.ascontiguousarray(
        xT_d.reshape(KT, 128, NTB, TB).transpose(2, 1, 0, 3))

    # RoPE tables (per-batch period; first 1024 tokens cover all)
    invf = 1.0 / (ROPE_BASE ** (np.arange(0, DH, 2, dtype=np.float32) / DH))
    tt = np.arange(T, dtype=np.float32)
    ang = tt[None, :] * invf[:, None]                        # [32, T]
    cos1 = np.cos(ang).astype(np.float32)
    sin1 = np.sin(ang).astype(np.float32)
    cos4 = np.tile(cos1, (4, 1))                             # [128, T]
    ssin4 = np.concatenate([-sin1, sin1, -sin1, sin1], axis=0)
    cs_host = np.ascontiguousarray(cos4.reshape(128, 2, TB).astype(BF))
    sn_host = np.ascontiguousarray(ssin4.reshape(128, 2, TB).astype(BF))

    # w_out layout: (hp, nh, p, i, t) = w_out[i*256 + hp*128 + p, nh*TB+t]
    wo_host = np.ascontiguousarray(
        w_out.reshape(8, 2, 128, 4, TB).transpose(1, 3, 2, 0, 4).astype(BF))

    in_maps = []
    for c in range(NCORES):
        heads = [4 * c + i for i in range(HL)]

        def deint(h, base):
            cols = np.arange(h * DH, (h + 1) * DH)
            return np.concatenate([base + cols[0::2], base + cols[1::2]])

        qk_cols = np.concatenate(
            [deint(heads[0], 0), deint(heads[1], 0),
             deint(heads[2], 0), deint(heads[3], 0),
             deint(heads[0], D), deint(heads[1], D),
             deint(heads[2], D), deint(heads[3], D)])
        v_cols = np.concatenate([2 * D + np.arange(h * DH, (h + 1) * DH)
                                 for h in heads])
        # [2048, 512] -> [128, 64, 128]
        wqk_host = np.ascontiguousarray(
            w_qkv[:, qk_cols].reshape(KT, 128, 4, 128)
            .transpose(1, 0, 2, 3).reshape(128, KT * 4, 128).astype(BF))
        wv_host = np.ascontiguousarray(
            w_qkv[:, v_cols].reshape(KT, 128, 256)
            .transpose(1, 0, 2).astype(BF))
        wr_host = np.ascontiguousarray(
            w_router[c * DSL:(c + 1) * DSL, :].reshape(2, 128, H)
            .transpose(1, 0, 2))
        sel_np = np.zeros((H, 97), dtype=np.float32)
        for l in range(HL):
            sel_np[4 * c + l, 32 * l] = 1.0
        in_maps.append({
            "xT": xT_host,
            "xsl": np.ascontiguousarray(x2[:, c * DSL:(c + 1) * DSL].T),
            "wqk": wqk_host,
            "wv": wv_host,
            "wr": wr_host,
            "wo": wo_host,
            "cs": cs_host,
            "sn": sn_host,
            "sel": sel_np,
        })
    return in_maps


def run(x, w_router, w_qkv, w_out, trace=False, debug=False):
    if debug:
        nc = _build(debug=True)
    else:
        nc = _get_nc()
    in_maps = _host_inputs(x, w_router, w_qkv, w_out)
    res = run_bass_kernel_spmd(nc, in_maps, core_ids=list(range(NCORES)), trace=trace)
    shards = [res.results[c]["out"] for c in range(NCORES)]
    full = np.concatenate(shards, axis=0).reshape(B, T, D).astype(np.float32)
    return full, res


def kernel(x, w_router, w_qkv, w_out):
    full, _ = run(x, w_router, w_qkv, w_out, trace=False)
    return full

